# revision 9
# baseline (speedup 1.0000x reference)
"""Trainium2 Bass kernel for nn_ClassChannelAttention.

Computes: out = x * scale[None, :, None, None] where
  scale[c] = sum_k softmax(channel_attention, axis=-1)[k, c]

Sharding: data-parallel over batch B=16 across 8 cores (2 batches/core);
channel_attention (150, 768) replicated to every core. The softmax+class-sum
is tiny and recomputed on each core (no collectives needed).

Per-core layout: x shard viewed as (1536, 4096) — 128 channels on SBUF
partitions, 4096 spatial elements on the free dim, 12 tiles. Each tile is
scaled by a per-partition scalar (DVE tensor_scalar_mul) drawn from a
(128, 6) scale tile whose column k holds channels [k*128, (k+1)*128).

The cross-partition class-sum is done with tiny PE matmuls:
  scale_block_k[128, 1] = attn_norm[:, k*128:(k+1)*128].T @ ones[150, 1]
which lands the result directly with channels on partitions (no transpose).
"""

import numpy as np

import concourse.bacc as bacc
import concourse.mybir as mybir
import concourse.tile as tile
from concourse import bass_utils

N_CORES = 8
B, C, H, W = 16, 768, 64, 64
K_CLS = 150
B_SH = B // N_CORES          # 2 batches per core
F = H * W                    # 4096
ROWS = B_SH * C              # 1536
P = 128
N_BLK = C // P               # 6 channel blocks
N_ROWBLK = ROWS // P         # 12 row blocks per core
COLS = 4096                  # free-dim tile width (divides F)
N_COL = F // COLS            # column splits per row block
X_BUFS = 8                   # SBUF ring depth for the main x tiles

_module_cache = {}


def _body(tc, out, x, ca):
    nc = tc.nc
    f32 = mybir.dt.float32
    Exp = mybir.ActivationFunctionType.Exp

    with (
        tc.tile_pool(name="attn", bufs=2) as attn_pool,
        tc.tile_pool(name="small", bufs=1) as small,
        tc.tile_pool(name="psum", bufs=1, space="PSUM") as psum_pool,
        tc.tile_pool(name="xt", bufs=X_BUFS) as xpool,
    ):
        ones = small.tile([P, 1], f32)
        nc.vector.memset(ones, 1.0)

        scale = small.tile([P, N_BLK], f32)
        psums = [
            psum_pool.tile([P, 1], f32, name=f"ps{k}", tag=f"ps{k}")
            for k in range(N_BLK)
        ]

        # Softmax over channels per class; classes on partitions (128 + 22).
        row_splits = [(0, 128), (128, K_CLS - 128)]
        for idx, (r0, rn) in enumerate(row_splits):
            at = attn_pool.tile([P, C], f32, tag="attn")
            nc.sync.dma_start(out=at[:rn], in_=ca[r0 : r0 + rn])
            negm = attn_pool.tile([P, 1], f32, tag="negm")
            nc.vector.reduce_max(
                out=negm[:rn], in_=at[:rn], axis=mybir.AxisListType.X, negate=True
            )
            e = attn_pool.tile([P, C], f32, tag="e")
            s = attn_pool.tile([P, 1], f32, tag="s")
            # e = exp(at - max); s = per-class row sum of e (fused accum).
            nc.scalar.activation(
                out=e[:rn], in_=at[:rn], func=Exp, bias=negm[:rn], accum_out=s[:rn]
            )
            r = attn_pool.tile([P, 1], f32, tag="r")
            nc.vector.reciprocal(out=r[:rn], in_=s[:rn])
            nc.vector.tensor_scalar_mul(e[:rn], e[:rn], r[:rn])
            # Class-sum into channel-on-partition layout, one block at a time.
            for k in range(N_BLK):
                nc.tensor.matmul(
                    psums[k],
                    lhsT=e[:rn, k * P : (k + 1) * P],
                    rhs=ones[:rn],
                    start=(idx == 0),
                    stop=(idx == len(row_splits) - 1),
                )
        for k in range(N_BLK):
            nc.scalar.copy(out=scale[:, k : k + 1], in_=psums[k])

        # Main scaled copy: (1536, 4096) in 12 tiles of (128, 4096).
        xf = x.rearrange("b c h w -> (b c) (h w)")
        of = out.rearrange("b c h w -> (b c) (h w)")
        # Loads go through the Sync HWDGE queue, stores through the Scalar
        # HWDGE queue — two independent FIFOs so store backlog never stalls
        # the next load's descriptors (single-queue convoy effect), and HBM
        # reads/writes stream concurrently.
        #
        # Each DMA is chunked to <=15 partitions: HWDGE deals packet j of a
        # DMA to SDMA engine 64+(j%16) restarting at 64 per DMA, and engine
        # 79 is ~13% slower than the rest (it also hosts the queue rings).
        # 15-partition DMAs keep all traffic on the 15 fast engines.
        chunks = [(c * 15, min(15, P - c * 15)) for c in range((P + 14) // 15)]
        for i in range(N_ROWBLK * N_COL):
            rb, cb = i // N_COL, i % N_COL
            k = rb % N_BLK
            r0 = rb * P
            cols = slice(cb * COLS, (cb + 1) * COLS)
            xt = xpool.tile([P, COLS], f32, tag="xt")
            for p0, pn in chunks:
                nc.sync.dma_start(
                    out=xt[p0 : p0 + pn], in_=xf[r0 + p0 : r0 + p0 + pn, cols]
                )
            nc.vector.tensor_scalar_mul(xt, xt, scale[:, k : k + 1])
            for p0, pn in chunks:
                nc.scalar.dma_start(
                    out=of[r0 + p0 : r0 + p0 + pn, cols], in_=xt[p0 : p0 + pn]
                )


def _get_module():
    if "nc" in _module_cache:
        return _module_cache["nc"]
    nc = bacc.Bacc(
        "TRN2", target_bir_lowering=False, debug=False, enable_asserts=False
    )
    x = nc.dram_tensor(
        "x", (B_SH, C, H, W), mybir.dt.float32, kind="ExternalInput"
    ).ap()
    ca = nc.dram_tensor(
        "channel_attention", (K_CLS, C), mybir.dt.float32, kind="ExternalInput"
    ).ap()
    out = nc.dram_tensor(
        "out", (B_SH, C, H, W), mybir.dt.float32, kind="ExternalOutput"
    ).ap()
    with tile.TileContext(nc) as tc:
        _body(tc, out, x, ca)
    nc.compile()
    _module_cache["nc"] = nc
    return nc


def _run(x, channel_attention, **spmd_kwargs):
    x = np.ascontiguousarray(np.asarray(x, dtype=np.float32))
    ca = np.ascontiguousarray(np.asarray(channel_attention, dtype=np.float32))
    assert x.shape == (B, C, H, W), x.shape
    assert ca.shape == (K_CLS, C), ca.shape
    nc = _get_module()
    in_maps = [
        {"x": x[i * B_SH : (i + 1) * B_SH], "channel_attention": ca}
        for i in range(N_CORES)
    ]
    res = bass_utils.run_bass_kernel_spmd(
        nc, in_maps, core_ids=list(range(N_CORES)), **spmd_kwargs
    )
    out = np.concatenate([r["out"] for r in res.results], axis=0)
    return out, res


def kernel(x, channel_attention):
    out, _ = _run(x, channel_attention)
    return out


# revision 12
# speedup vs baseline: 1.8456x; 1.8456x over previous
"""Trainium2 Bass kernel for nn_ClassChannelAttention.

Computes: out = x * scale[None, :, None, None] where
  scale[c] = sum_k softmax(channel_attention, axis=-1)[k, c]

Sharding: data-parallel over batch B=16 across 8 cores (2 batches/core);
channel_attention (150, 768) replicated to every core. The softmax+class-sum
is tiny and recomputed on each core (no collectives needed).

Per-core layout: x shard viewed as (1536, 4096) — 128 channels on SBUF
partitions, 4096 spatial elements on the free dim, 12 tiles. Each tile is
scaled by a per-partition scalar (DVE tensor_scalar_mul) drawn from a
(128, 6) scale tile whose column k holds channels [k*128, (k+1)*128).

The cross-partition class-sum is done with tiny PE matmuls:
  scale_block_k[128, 1] = attn_norm[:, k*128:(k+1)*128].T @ ones[150, 1]
which lands the result directly with channels on partitions (no transpose).
"""

import numpy as np

import concourse.bacc as bacc
import concourse.mybir as mybir
import concourse.tile as tile
from concourse import bass_utils

N_CORES = 8
B, C, H, W = 16, 768, 64, 64
K_CLS = 150
B_SH = B // N_CORES          # 2 batches per core
F = H * W                    # 4096
ROWS = B_SH * C              # 1536
P = 128
N_BLK = C // P               # 6 channel blocks
F2 = 2 * F                   # 8192: two channel-rows merged -> 32 KiB DMA rows
ROWS2 = ROWS // 2            # 768 rows in the merged view
N_TILES = ROWS2 // P         # 6 tiles of (128, 8192) per core
X_BUFS = 4                   # SBUF ring depth for the main x tiles

_module_cache = {}


def _body(tc, out, x, ca):
    nc = tc.nc
    f32 = mybir.dt.float32
    Exp = mybir.ActivationFunctionType.Exp

    with (
        tc.tile_pool(name="attn", bufs=2) as attn_pool,
        tc.tile_pool(name="small", bufs=1) as small,
        tc.tile_pool(name="psum", bufs=1, space="PSUM") as psum_pool,
        tc.tile_pool(name="xt", bufs=X_BUFS) as xpool,
    ):
        ones = small.tile([P, 1], f32)
        nc.vector.memset(ones, 1.0)

        # scale columns 0..2 = even channels at offset 256j (j = tile % 3),
        # columns 3..5 = odd channels: scale[:, j][p] = sum-softmax over
        # channel 256j + 2p (+1 for odd).
        scale = small.tile([P, 2 * 3], f32)
        psums = [
            psum_pool.tile([P, 1], f32, name=f"ps{k}", tag=f"ps{k}")
            for k in range(6)
        ]

        # Softmax over channels per class; classes on partitions (128 + 22).
        row_splits = [(0, 128), (128, K_CLS - 128)]
        for idx, (r0, rn) in enumerate(row_splits):
            at = attn_pool.tile([P, C], f32, tag="attn")
            nc.sync.dma_start(out=at[:rn], in_=ca[r0 : r0 + rn])
            negm = attn_pool.tile([P, 1], f32, tag="negm")
            nc.vector.reduce_max(
                out=negm[:rn], in_=at[:rn], axis=mybir.AxisListType.X, negate=True
            )
            e = attn_pool.tile([P, C], f32, tag="e")
            s = attn_pool.tile([P, 1], f32, tag="s")
            # e = exp(at - max); s = per-class row sum of e (fused accum).
            nc.scalar.activation(
                out=e[:rn], in_=at[:rn], func=Exp, bias=negm[:rn], accum_out=s[:rn]
            )
            r = attn_pool.tile([P, 1], f32, tag="r")
            nc.vector.reciprocal(out=r[:rn], in_=s[:rn])
            nc.vector.tensor_scalar_mul(e[:rn], e[:rn], r[:rn])
            # Class-sum into channel-on-partition layout via tiny matmuls.
            # e viewed as (cls, 3 offsets, 128 channel-pairs, even/odd).
            e_r = e.rearrange("k (c a two) -> k c a two", c=3, two=2)
            for j in range(3):
                for parity in range(2):
                    nc.tensor.matmul(
                        psums[3 * parity + j],
                        lhsT=e_r[:rn, j, :, parity],
                        rhs=ones[:rn],
                        start=(idx == 0),
                        stop=(idx == len(row_splits) - 1),
                    )
        for k in range(6):
            nc.scalar.copy(out=scale[:, k : k + 1], in_=psums[k])

        # Main scaled copy: shard viewed as (768, 8192) — each partition row
        # carries two consecutive channel rows (32 KiB contiguous). 32 KiB
        # DMA rows matter: with 16 KiB packets, SDMA engine 79 (which also
        # hosts the HWDGE queue rings) runs ~13% slower than the other 15
        # and becomes the straggler; at 32 KiB rows it runs at full rate.
        xf = x.rearrange("b c h w -> (b c) (h w)").rearrange(
            "(a two) f -> a (two f)", two=2
        )
        of = out.rearrange("b c h w -> (b c) (h w)").rearrange(
            "(a two) f -> a (two f)", two=2
        )
        # Loads on the Sync HWDGE queue, stores on the Scalar HWDGE queue —
        # two independent FIFOs so reads and writes stream concurrently.
        for i in range(N_TILES):
            j = i % 3
            rows = slice(i * P, (i + 1) * P)
            xt = xpool.tile([P, F2], f32, tag="xt")
            nc.sync.dma_start(out=xt, in_=xf[rows])
            # even half: channels 256j + 2p ; odd half: +1
            nc.vector.tensor_scalar_mul(
                xt[:, 0:F], xt[:, 0:F], scale[:, j : j + 1]
            )
            nc.vector.tensor_scalar_mul(
                xt[:, F:F2], xt[:, F:F2], scale[:, 3 + j : 4 + j]
            )
            nc.scalar.dma_start(out=of[rows], in_=xt)


def _get_module():
    if "nc" in _module_cache:
        return _module_cache["nc"]
    nc = bacc.Bacc(
        "TRN2", target_bir_lowering=False, debug=False, enable_asserts=False
    )
    x = nc.dram_tensor(
        "x", (B_SH, C, H, W), mybir.dt.float32, kind="ExternalInput"
    ).ap()
    ca = nc.dram_tensor(
        "channel_attention", (K_CLS, C), mybir.dt.float32, kind="ExternalInput"
    ).ap()
    out = nc.dram_tensor(
        "out", (B_SH, C, H, W), mybir.dt.float32, kind="ExternalOutput"
    ).ap()
    with tile.TileContext(nc) as tc:
        _body(tc, out, x, ca)
    nc.compile()
    _module_cache["nc"] = nc
    return nc


def _run(x, channel_attention, **spmd_kwargs):
    x = np.ascontiguousarray(np.asarray(x, dtype=np.float32))
    ca = np.ascontiguousarray(np.asarray(channel_attention, dtype=np.float32))
    assert x.shape == (B, C, H, W), x.shape
    assert ca.shape == (K_CLS, C), ca.shape
    nc = _get_module()
    in_maps = [
        {"x": x[i * B_SH : (i + 1) * B_SH], "channel_attention": ca}
        for i in range(N_CORES)
    ]
    res = bass_utils.run_bass_kernel_spmd(
        nc, in_maps, core_ids=list(range(N_CORES)), **spmd_kwargs
    )
    out = np.concatenate([r["out"] for r in res.results], axis=0)
    return out, res


def kernel(x, channel_attention):
    out, _ = _run(x, channel_attention)
    return out
